# revision 20
# baseline (speedup 1.0000x reference)
"""Trainium2 Bass kernel for nn_DynamicImagePrimalDualNN.

T=128 primal-dual iterations over (2,1,160,160,32) with circular FD stencils.

Distribution: mb(2) x x-slabs(4) = 8 cores (ranks 0-3 = image 0, 4-7 = image
1; slab = rank%4). y and t stay core-local. One AllGather of the two edge
xbar planes per iteration, hidden under compute; qx kept on a 41-wide
overlap slab so the divergence is fully local.

Per-core layout: partitions p = (y%4)*32 + t; free = (x_slot, yb).

Rescaled primal X = x/c2 (c2 = ta*sig) so the x-passthrough and all
divergence stationaries are exact +-1 patterns; duals Q = q/sig and
mt = p/sig stay baseline-scaled. c2 appears only on the forward-gradient
coefficients (bf16-rounded stencils / fp32 stt immediates - a benign
increment-coefficient perturbation, non-compounding):
  mt' = a*mt + (a*c2)*Xbar - cxn        (cxn = a*xnoisy, a = 1/(1+sig))
  Q'  = clip(Q + c2*grad(Xbar), lam/sig)
  X1  = X0 - mt' - div(Q')
  Xbar'= (1+th)*X1 - th*X0
  out = c2*X1 (final scale host-side)

Engine split per iteration (approx): PE 48 matmuls (stencils + psum
accumulation; X0 injected as float32r at full rate), DVE clips/p-phase,
ACT psum->sbuf copies (X1/XBP/ZN/qst + pads), DVE the rest (GPSIMD
elementwise is not supported by this walrus build). X1 state is float32r (13-bit mantissa rounding per iteration,
passthrough via exact *1.0 is lossless).
"""

import math
from contextlib import ExitStack
from functools import lru_cache

import numpy as np

import concourse.bass as bass
import concourse.tile as tile
from concourse import bacc, mybir
from concourse.bass_utils import run_bass_kernel_spmd

F32 = mybir.dt.float32
F32R = mybir.dt.float32r
BF = mybir.dt.bfloat16
AX = mybir.AluOpType
ACTF = mybir.ActivationFunctionType

T_ITERS = 128
TRACE = False
_LAST_RESULTS = None
NXS = 40          # x-slab width per core
NYB = 40          # y blocks (y = 4*yb + my)
NCH = 10          # x-chunk width for PSUM-bank-sized matmuls
GROUPS = [[0, 1, 2, 3], [4, 5, 6, 7]]


def _pidx(m, t):
    return m * 32 + t


def _stationaries():
    """(128,128) matrices W[p_in, p_out]; matmul computes out[i] = sum_k W[k,i] in[k]."""
    I = np.eye(128, dtype=np.float32)
    dy = -np.eye(128, dtype=np.float32)
    cy = np.zeros((128, 128), np.float32)
    dt = -np.eye(128, dtype=np.float32)
    dyh = -np.eye(128, dtype=np.float32)
    cyh = np.zeros((128, 128), np.float32)
    dth = -np.eye(128, dtype=np.float32)
    for t in range(32):
        for m in range(3):
            dy[_pidx(m + 1, t), _pidx(m, t)] += 1.0
        cy[_pidx(0, t), _pidx(3, t)] = 1.0
        for m in range(1, 4):
            dyh[_pidx(m - 1, t), _pidx(m, t)] += 1.0
        cyh[_pidx(3, t), _pidx(0, t)] = 1.0
        for m in range(4):
            dt[_pidx(m, (t + 1) % 32), _pidx(m, t)] += 1.0
            dth[_pidx(m, (t - 1) % 32), _pidx(m, t)] += 1.0
    return dict(w_i=I, w_ni=-I, w_dy=dy, w_cy=cy, w_dt=dt, w_dyh=dyh,
                w_cyh=cyh, w_dth=dth)


def to_dev(v):
    """(xs, 160y, 32t) -> (128, xs, 40yb) with p=(y%4)*32+t."""
    xs = v.shape[0]
    return np.ascontiguousarray(
        v.reshape(xs, NYB, 4, 32).transpose(2, 3, 0, 1).reshape(128, xs, NYB))


def from_dev(v):
    """(128, xs, 40yb) -> (xs, 160y, 32t)."""
    xs = v.shape[1]
    return np.ascontiguousarray(
        v.reshape(4, 32, xs, NYB).transpose(2, 3, 0, 1).reshape(xs, 160, 32))


def _build_nc(scalars, T=T_ITERS):
    a_, th, c2 = scalars
    nc = bacc.Bacc("TRN2", target_bir_lowering=False, debug=False,
                   num_devices=8)

    dp = {}
    dp["xb0"] = nc.dram_tensor("xb0", [128, NXS, NYB], BF,
                               kind="ExternalInput")
    dp["x00"] = nc.dram_tensor("x00", [128, NXS, NYB], F32R,
                               kind="ExternalInput")
    for name in ("z00", "m0", "cxn"):
        dp[name] = nc.dram_tensor(name, [128, NXS, NYB], BF,
                                  kind="ExternalInput")
    # x-channel lambda covers the 41-wide overlap slab
    for name in ("lamx", "nlamx"):
        dp[name] = nc.dram_tensor(name, [128, NXS + 1, NYB], BF,
                                  kind="ExternalInput")
    for name in ("lamy", "nlamy", "lamt", "nlamt"):
        dp[name] = nc.dram_tensor(name, [128, NXS, NYB], BF,
                                  kind="ExternalInput")
    # (128, 8, 40) combined one-hot mask over gathered slots, broadcast
    # along yb (slot = rank_in_group*2 + e; even slots -> hi, odd -> lo)
    dp["mskc"] = nc.dram_tensor("mskc", [128, 8, NYB], BF,
                                kind="ExternalInput")
    wnames = list(_stationaries().keys()) + ["w_ic2", "w_nic2"]
    for name in wnames:
        dp[name] = nc.dram_tensor(name, [128, 128], BF, kind="ExternalInput")
    dp["w_ni32r"] = nc.dram_tensor("w_ni32r", [128, 128], F32R,
                                   kind="ExternalInput")
    out_dram = nc.dram_tensor("out", [128, NXS, NYB], F32,
                              kind="ExternalOutput")

    with tile.TileContext(nc) as tc, ExitStack() as es:
        state = es.enter_context(tc.tile_pool(name="state", bufs=1))
        xpool = es.enter_context(tc.tile_pool(name="xp", bufs=2))
        zpool = es.enter_context(tc.tile_pool(name="zp", bufs=2))
        spool = es.enter_context(tc.tile_pool(name="scratch", bufs=2))
        dpool = es.enter_context(tc.tile_pool(name="dram", bufs=2,
                                              space="DRAM"))
        gpool = es.enter_context(tc.tile_pool(name="gath", bufs=2))
        psum_y = es.enter_context(
            tc.tile_pool(name="psum_y", bufs=2, space=bass.MemorySpace.PSUM))
        psum_t = es.enter_context(
            tc.tile_pool(name="psum_t", bufs=1, space=bass.MemorySpace.PSUM))
        psum_x = es.enter_context(
            tc.tile_pool(name="psum_x", bufs=3, space=bass.MemorySpace.PSUM))
        psum_q = es.enter_context(
            tc.tile_pool(name="psum_q", bufs=2, space=bass.MemorySpace.PSUM))

        # xbar: x slots 0=halo_lo, 1..40 real, 41=halo_hi; yb col 40 =
        # pad(yb0), col 41 unused (even stride keeps bf16 2x alignment)
        xbar = state.tile([128, NXS + 2, NYB + 2], BF, tag="xbar")
        # qx on the 41-wide overlap slab (col j = global x s-1+j), no halos
        qx = state.tile([128, NXS + 1, NYB], BF, tag="qx")
        # qy: yb col 0 = pad(yb39), real yb at cols 1..40, col 41 unused
        qy = state.tile([128, NXS, NYB + 2], BF, tag="qy")
        qt = state.tile([128, NXS, NYB], BF, tag="qt")
        m = state.tile([128, NXS, NYB], BF, tag="m")
        cxn = state.tile([128, NXS, NYB], BF, tag="cxn")
        lamx = state.tile([128, NXS + 1, NYB], BF, tag="lamx")
        nlamx = state.tile([128, NXS + 1, NYB], BF, tag="nlamx")
        lamy = state.tile([128, NXS, NYB], BF, tag="lamy")
        nlamy = state.tile([128, NXS, NYB], BF, tag="nlamy")
        lamt = state.tile([128, NXS, NYB], BF, tag="lamt")
        nlamt = state.tile([128, NXS, NYB], BF, tag="nlamt")
        mskc = state.tile([128, 8, NYB], BF, tag="mskc")
        W = {n: state.tile([128, 128], BF, tag=n, name=f"w_{n}")
             for n in wnames}
        Wx = state.tile([128, 128], F32R, tag="w_ni32r", name="w_ni32r")

        nc.sync.dma_start(xbar[:, 1:41, 0:40], dp["xb0"][:])
        x0 = xpool.tile([128, NXS, NYB], F32R, tag="x", name="x_init")
        nc.sync.dma_start(x0[:], dp["x00"][:])
        zt = zpool.tile([128, NXS, NYB], BF, tag="z", name="z_init")
        nc.sync.dma_start(zt[:], dp["z00"][:])
        nc.sync.dma_start(m[:], dp["m0"][:])
        nc.sync.dma_start(cxn[:], dp["cxn"][:])
        for nm, tl in (("lamx", lamx), ("nlamx", nlamx), ("lamy", lamy),
                       ("nlamy", nlamy), ("lamt", lamt), ("nlamt", nlamt),
                       ("mskc", mskc)):
            nc.sync.dma_start(tl[:], dp[nm][:])
        for n in wnames:
            nc.sync.dma_start(W[n][:], dp[n][:])
        nc.sync.dma_start(Wx[:], dp["w_ni32r"][:])
        nc.vector.memset(qx[:], 0.0)
        nc.vector.memset(qy[:], 0.0)
        nc.vector.memset(qt[:], 0.0)
        nc.vector.tensor_copy(xbar[:, 1:41, 40:41], xbar[:, 1:41, 0:1])

        def exchange(round_idx):
            """AG of my (first,last) xbar planes; returns gathered dram tile."""
            bin_ = dpool.tile([2, 128, NYB], BF, tag="bin",
                              name=f"bin{round_idx}")
            bout = dpool.tile([8, 128, NYB], BF, tag="bout",
                              name=f"bout{round_idx}")
            nc.sync.dma_start(bin_[0], xbar[:, 1, 0:40])
            nc.sync.dma_start(bin_[1], xbar[:, 40, 0:40])
            nc.gpsimd.collective_compute(
                "AllGather", AX.bypass, replica_groups=GROUPS,
                ins=[bin_[:]], outs=[bout[:]])
            return bout

        def recv(bout):
            """DMA gathered planes to SBUF; one masked multiply + one
            axis-reduce -> (hi, lo), then 2 small copies into xbar halos."""
            gath = gpool.tile([128, 8, NYB], BF, tag="gath")
            nc.sync.dma_start(gath[:], bout[:].transpose([1, 0, 2]))
            u = gpool.tile([128, 8, NYB], BF, tag="u")
            nc.vector.tensor_tensor(u[:], gath[:], mskc[:], AX.mult)
            h = gpool.tile([128, 2, NYB], BF, tag="h")
            # view u as [p, e(2), y(40), j(4)] (slot = 2j+e) and reduce j
            uv = u[:].rearrange("p (j e) y -> p e y j", j=4, e=2)
            with nc.allow_low_precision(reason="one-hot select, sum exact"):
                nc.vector.tensor_reduce(h[:], uv, mybir.AxisListType.X,
                                        AX.add)
            nc.scalar.copy(xbar[:, 41, 0:40], h[:, 0, :])
            nc.scalar.copy(xbar[:, 0, 0:40], h[:, 1, :])

        bout = exchange(0)

        ac2 = float(np.float32(a_) * np.float32(c2))
        CORD = (1, 0, 3, 2)

        for k in range(T):
            # --- p-phase early (DVE; only needs xbar) ---
            t1 = spool.tile([128, NXS, NYB], BF, tag="t1")
            nc.vector.tensor_scalar(t1[:], xbar[:, 1:41, 0:40], ac2,
                                    None, AX.mult)
            nc.vector.tensor_sub(t1[:], t1[:], cxn[:])
            nc.vector.tensor_scalar(m[:], m[:], a_, None, AX.mult)
            nc.vector.tensor_add(m[:], m[:], t1[:])

            # --- qy/qt psums on PE ---
            ps_y, ps_t = {}, {}
            for c in CORD:
                sl = slice(1 + NCH * c, 1 + NCH * (c + 1))
                slq = slice(NCH * c, NCH * (c + 1))
                psy = psum_y.tile([128, NCH, NYB], F32, tag="psy",
                                  name=f"psy{k}_{c}")
                nc.tensor.matmul(psy[:], W["w_i"][:], qy[:, slq, 1:41],
                                 start=True, stop=False)
                nc.tensor.matmul(psy[:], W["w_dy"][:], xbar[:, sl, 0:40],
                                 start=False, stop=False)
                nc.tensor.matmul(psy[:], W["w_cy"][:], xbar[:, sl, 1:41],
                                 start=False, stop=True)
                ps_y[c] = psy
                pst = psum_t.tile([128, NCH, NYB], F32, tag="pst",
                                  name=f"pst{k}_{c}")
                nc.tensor.matmul(pst[:], W["w_i"][:], qt[:, slq, :],
                                 start=True, stop=False)
                nc.tensor.matmul(pst[:], W["w_dt"][:], xbar[:, sl, 0:40],
                                 start=False, stop=True)
                ps_t[c] = pst

            # --- qx via PE psums (c2-scaled x-shift stencil, incl halo
            # cols); chunks of 10|10|10|11 over the 41-wide slab. recv-
            # dependent chunks (0: halo_lo, 3: halo_hi) run last ---
            recv(bout)
            ps_q = {}
            for c in (1, 2, 0, 3):
                lo = 10 * c
                w = 11 if c == 3 else 10
                psq = psum_q.tile([128, 11, NYB], F32, tag="psq",
                                  name=f"psq{k}_{c}")
                pv = psq[:, 0:w, :]
                nc.tensor.matmul(pv, W["w_i"][:], qx[:, lo:lo + w, :],
                                 start=True, stop=False)
                nc.tensor.matmul(pv, W["w_ic2"][:],
                                 xbar[:, lo + 1:lo + w + 1, 0:40],
                                 start=False, stop=False)
                nc.tensor.matmul(pv, W["w_nic2"][:],
                                 xbar[:, lo:lo + w, 0:40],
                                 start=False, stop=True)
                ps_q[c] = (psq, lo, w)
            for c in (1, 2, 0, 3):
                psq, lo, w = ps_q[c]
                nc.vector.tensor_tensor(qx[:, lo:lo + w, :], psq[:, 0:w, :],
                                        nlamx[:, lo:lo + w, :], AX.max)
                nc.vector.tensor_tensor(qx[:, lo:lo + w, :],
                                        qx[:, lo:lo + w, :],
                                        lamx[:, lo:lo + w, :], AX.min)

            # --- per-chunk clips, then per-chunk x-psum + xbar writes:
            # pipelined so the PE restarts as soon as chunk deps land ---
            x1 = xpool.tile([128, NXS, NYB], F32R, tag="x", name=f"x{k}")
            xbp = spool.tile([128, NXS, NYB], BF, tag="xbp")
            zn = zpool.tile([128, NXS, NYB], BF, tag="z", name=f"z{k}")
            qsy = spool.tile([128, NXS, NYB], BF, tag="qsy")
            qst = spool.tile([128, NXS, NYB], BF, tag="qst")
            for c in CORD:
                slq = slice(NCH * c, NCH * (c + 1))
                nc.scalar.activation(qsy[:, slq, :], ps_y[c][:], ACTF.Copy)
                nc.scalar.activation(qst[:, slq, :], ps_t[c][:], ACTF.Copy)
                nc.vector.tensor_tensor(qy[:, slq, 1:41], qsy[:, slq, :],
                                        nlamy[:, slq, :], AX.max)
                nc.vector.tensor_tensor(qy[:, slq, 1:41], qy[:, slq, 1:41],
                                        lamy[:, slq, :], AX.min)
                nc.scalar.copy(qy[:, slq, 0:1], qy[:, slq, 40:41])
                nc.vector.tensor_tensor(qt[:, slq, :], qst[:, slq, :],
                                        nlamt[:, slq, :], AX.max)
                nc.vector.tensor_tensor(qt[:, slq, :], qt[:, slq, :],
                                        lamt[:, slq, :], AX.min)

            for c in CORD:
                slq = slice(NCH * c, NCH * (c + 1))          # qx[x-1]
                slq1 = slice(NCH * c + 1, NCH * (c + 1) + 1)  # qx[x]
                sl = slice(1 + NCH * c, 1 + NCH * (c + 1))
                ps = psum_x.tile([128, NCH, NYB], F32, tag="psx",
                               name=f"psx{k}_{c}")
                nc.tensor.matmul(ps[:], Wx[:], x0[:, slq, :],
                                 start=True, stop=False)
                nc.tensor.matmul(ps[:], W["w_dyh"][:], qy[:, slq, 1:41],
                                 start=False, stop=False)
                nc.tensor.matmul(ps[:], W["w_cyh"][:], qy[:, slq, 0:40],
                                 start=False, stop=False)
                nc.tensor.matmul(ps[:], W["w_dth"][:], qt[:, slq, :],
                                 start=False, stop=False)
                nc.tensor.matmul(ps[:], W["w_i"][:], qx[:, slq, :],
                                 start=False, stop=False)
                nc.tensor.matmul(ps[:], W["w_ni"][:], qx[:, slq1, :],
                                 start=False, stop=False)
                nc.tensor.matmul(ps[:], W["w_i"][:], m[:, slq, :],
                                 start=False, stop=True)
                if k < T - 1:
                    nc.scalar.activation(xbp[:, slq, :], ps[:], ACTF.Copy,
                                         scale=-(1.0 + th))
                    # xbar chunk + its pad-col slice ready immediately
                    nc.vector.tensor_sub(xbar[:, sl, 0:40], xbp[:, slq, :],
                                         zt[:, slq, :])
                    nc.scalar.copy(xbar[:, sl, 40:41], xbar[:, sl, 0:1])
                    nc.scalar.activation(zn[:, slq, :], ps[:], ACTF.Copy,
                                         scale=-th)
                nc.scalar.activation(x1[:, slq, :], ps[:], ACTF.Copy,
                                     scale=-1.0)
                if k < T - 1 and c == 3:
                    bout = exchange(k + 1)
            x0 = x1
            zt = zn

        # out = c2 * X1 (c2 folded host-side via out scaling input? no:
        # scale here with ACT once)
        nc.sync.dma_start(out_dram[:], x0[:].bitcast(F32))

    nc.compile()
    return nc


@lru_cache(maxsize=4)
def _compiled(scalars, T):
    return _build_nc(scalars, T)


def _make_in_maps(x, lambda_map, scalars, sig):
    import ml_dtypes
    bf = ml_dtypes.bfloat16
    stats = _stationaries()
    a_, th, c2 = scalars
    rc2 = 1.0 / np.float32(c2)
    in_maps = []
    for rank in range(8):
        mbi, pos = rank // 4, rank % 4
        s = pos * NXS
        xs = slice(s, s + NXS)
        xn = np.ascontiguousarray(x[mbi, 0, xs]).astype(np.float32)
        X = xn * rc2
        lam = lambda_map[mbi].astype(np.float32) / np.float32(sig)
        # x-channel lambda on the 41-wide overlap slab [s-1, s+40)
        idx = [(s - 1 + j) % 160 for j in range(NXS + 1)]
        lx = lam[0][idx]
        nxt, prv = (pos + 1) % 4, (pos - 1) % 4
        mc = np.zeros((128, 8, NYB), np.float32)
        mc[:, 2 * nxt, :] = 1.0      # next's first plane -> halo_hi (even)
        mc[:, 2 * prv + 1, :] = 1.0  # prev's last plane  -> halo_lo (odd)
        mm = dict(
            xb0=to_dev(X).astype(bf),
            x00=to_dev(X),
            z00=to_dev(np.float32(th) * X).astype(bf),
            m0=to_dev(xn / np.float32(sig)).astype(bf),
            cxn=to_dev(np.float32(a_) * xn).astype(bf),
            lamx=to_dev(lx).astype(bf), nlamx=to_dev(-lx).astype(bf),
            lamy=to_dev(lam[1][xs]).astype(bf),
            nlamy=to_dev(-lam[1][xs]).astype(bf),
            lamt=to_dev(lam[2][xs]).astype(bf),
            nlamt=to_dev(-lam[2][xs]).astype(bf),
            mskc=mc.astype(bf),
        )
        for k2, v in stats.items():
            # forward y/t stencils carry the c2 gradient coefficient
            if k2 in ("w_dy", "w_cy", "w_dt"):
                v = v * np.float32(c2)
            mm[k2] = v.astype(bf)
        eye = np.eye(128, dtype=np.float32)
        mm["w_ic2"] = (np.float32(c2) * eye).astype(bf)
        mm["w_nic2"] = (-np.float32(c2) * eye).astype(bf)
        mm["w_ni32r"] = -np.eye(128, dtype=np.float32)
        in_maps.append(mm)
    return in_maps


def kernel(x, lambda_map, tau, sigma, theta):
    x = np.asarray(x, dtype=np.float32)
    lambda_map = np.asarray(lambda_map, dtype=np.float32)
    L = math.sqrt(13.0)
    sig = float(1.0 / (1.0 + math.exp(-float(np.asarray(sigma)[0])))) / L
    ta = float(1.0 / (1.0 + math.exp(-float(np.asarray(tau)[0])))) / L
    th = float(1.0 / (1.0 + math.exp(-float(np.asarray(theta)[0]))))
    a_ = 1.0 / (1.0 + sig)
    c2 = ta * sig
    scalars = tuple(float(np.float32(v)) for v in (a_, th, c2))

    nc = _compiled(scalars, T_ITERS)
    in_maps = _make_in_maps(x, lambda_map, scalars, sig)
    res = run_bass_kernel_spmd(nc, in_maps, core_ids=list(range(8)),
                               trace=TRACE)
    global _LAST_RESULTS
    _LAST_RESULTS = res

    out = np.zeros((2, 1, 160, 160, 32), np.float32)
    for rank in range(8):
        mbi, pos = rank // 4, rank % 4
        s = pos * NXS
        out[mbi, 0, s:s + NXS] = from_dev(
            res.results[rank]["out"]) * np.float32(c2)
    return out


# revision 21
# speedup vs baseline: 1.0605x; 1.0605x over previous
"""Trainium2 Bass kernel for nn_DynamicImagePrimalDualNN.

T=128 primal-dual iterations over (2,1,160,160,32) with circular FD stencils.

Distribution: mb(2) x x-slabs(4) = 8 cores (ranks 0-3 = image 0, 4-7 = image
1; slab = rank%4). y and t stay core-local. One AllGather of the two edge
xbar planes per iteration, hidden under compute; qx kept on a 41-wide
overlap slab so the divergence is fully local.

Per-core layout: partitions p = (y%4)*32 + t; free = (x_slot, yb).

Rescaled primal X = x/c2 (c2 = ta*sig) so the x-passthrough and all
divergence stationaries are exact +-1 patterns; duals Q = q/sig and
mt = p/sig stay baseline-scaled. c2 appears only on the forward-gradient
coefficients (bf16-rounded stencils / fp32 stt immediates - a benign
increment-coefficient perturbation, non-compounding):
  mt' = a*mt + (a*c2)*Xbar - cxn        (cxn = a*xnoisy, a = 1/(1+sig))
  Q'  = clip(Q + c2*grad(Xbar), lam/sig)
  X1  = X0 - mt' - div(Q')
  Xbar'= (1+th)*X1 - th*X0
  out = c2*X1 (final scale host-side)

Engine split per iteration (approx): PE 48 matmuls (stencils + psum
accumulation; X0 injected as float32r at full rate), DVE clips/p-phase,
ACT psum->sbuf copies (X1/XBP/ZN/qst + pads), DVE the rest (GPSIMD
elementwise is not supported by this walrus build). X1 state is float32r (13-bit mantissa rounding per iteration,
passthrough via exact *1.0 is lossless).
"""

import math
from contextlib import ExitStack
from functools import lru_cache

import numpy as np

import concourse.bass as bass
import concourse.tile as tile
from concourse import bacc, mybir
from concourse.bass_utils import run_bass_kernel_spmd

F32 = mybir.dt.float32
F32R = mybir.dt.float32r
BF = mybir.dt.bfloat16
AX = mybir.AluOpType
ACTF = mybir.ActivationFunctionType

T_ITERS = 128
TRACE = False
_LAST_RESULTS = None
NXS = 40          # x-slab width per core
NYB = 40          # y blocks (y = 4*yb + my)
NCH = 10          # x-chunk width for PSUM-bank-sized matmuls
GROUPS = [[0, 1, 2, 3], [4, 5, 6, 7]]


def _pidx(m, t):
    return m * 32 + t


def _stationaries():
    """(128,128) matrices W[p_in, p_out]; matmul computes out[i] = sum_k W[k,i] in[k]."""
    I = np.eye(128, dtype=np.float32)
    dy = -np.eye(128, dtype=np.float32)
    cy = np.zeros((128, 128), np.float32)
    dt = -np.eye(128, dtype=np.float32)
    dyh = -np.eye(128, dtype=np.float32)
    cyh = np.zeros((128, 128), np.float32)
    dth = -np.eye(128, dtype=np.float32)
    for t in range(32):
        for m in range(3):
            dy[_pidx(m + 1, t), _pidx(m, t)] += 1.0
        cy[_pidx(0, t), _pidx(3, t)] = 1.0
        for m in range(1, 4):
            dyh[_pidx(m - 1, t), _pidx(m, t)] += 1.0
        cyh[_pidx(3, t), _pidx(0, t)] = 1.0
        for m in range(4):
            dt[_pidx(m, (t + 1) % 32), _pidx(m, t)] += 1.0
            dth[_pidx(m, (t - 1) % 32), _pidx(m, t)] += 1.0
    return dict(w_i=I, w_ni=-I, w_dy=dy, w_cy=cy, w_dt=dt, w_dyh=dyh,
                w_cyh=cyh, w_dth=dth)


def to_dev(v):
    """(xs, 160y, 32t) -> (128, xs, 40yb) with p=(y%4)*32+t."""
    xs = v.shape[0]
    return np.ascontiguousarray(
        v.reshape(xs, NYB, 4, 32).transpose(2, 3, 0, 1).reshape(128, xs, NYB))


def from_dev(v):
    """(128, xs, 40yb) -> (xs, 160y, 32t)."""
    xs = v.shape[1]
    return np.ascontiguousarray(
        v.reshape(4, 32, xs, NYB).transpose(2, 3, 0, 1).reshape(xs, 160, 32))


def _build_nc(scalars, T=T_ITERS):
    a_, th, c2 = scalars
    nc = bacc.Bacc("TRN2", target_bir_lowering=False, debug=False,
                   num_devices=8)

    dp = {}
    dp["xb0"] = nc.dram_tensor("xb0", [128, NXS, NYB], BF,
                               kind="ExternalInput")
    dp["x00"] = nc.dram_tensor("x00", [128, NXS, NYB], F32R,
                               kind="ExternalInput")
    for name in ("z00", "m0", "cxn"):
        dp[name] = nc.dram_tensor(name, [128, NXS, NYB], BF,
                                  kind="ExternalInput")
    # x-channel lambda covers the 41-wide overlap slab
    for name in ("lamx", "nlamx"):
        dp[name] = nc.dram_tensor(name, [128, NXS + 1, NYB], BF,
                                  kind="ExternalInput")
    for name in ("lamy", "nlamy", "lamt", "nlamt"):
        dp[name] = nc.dram_tensor(name, [128, NXS, NYB], BF,
                                  kind="ExternalInput")
    # (128, 8, 40) combined one-hot mask over gathered slots, broadcast
    # along yb (slot = rank_in_group*2 + e; even slots -> hi, odd -> lo)
    dp["mskc"] = nc.dram_tensor("mskc", [128, 8, NYB], BF,
                                kind="ExternalInput")
    wnames = list(_stationaries().keys()) + ["w_ic2", "w_nic2"]
    for name in wnames:
        dp[name] = nc.dram_tensor(name, [128, 128], BF, kind="ExternalInput")
    dp["w_ni32r"] = nc.dram_tensor("w_ni32r", [128, 128], F32R,
                                   kind="ExternalInput")
    out_dram = nc.dram_tensor("out", [128, NXS, NYB], F32,
                              kind="ExternalOutput")

    with tile.TileContext(nc) as tc, ExitStack() as es:
        state = es.enter_context(tc.tile_pool(name="state", bufs=1))
        xpool = es.enter_context(tc.tile_pool(name="xp", bufs=2))
        zpool = es.enter_context(tc.tile_pool(name="zp", bufs=2))
        spool = es.enter_context(tc.tile_pool(name="scratch", bufs=2))
        dpool = es.enter_context(tc.tile_pool(name="dram", bufs=2,
                                              space="DRAM"))
        gpool = es.enter_context(tc.tile_pool(name="gath", bufs=2))
        psum_y = es.enter_context(
            tc.tile_pool(name="psum_y", bufs=2, space=bass.MemorySpace.PSUM))
        psum_t = es.enter_context(
            tc.tile_pool(name="psum_t", bufs=1, space=bass.MemorySpace.PSUM))
        psum_x = es.enter_context(
            tc.tile_pool(name="psum_x", bufs=3, space=bass.MemorySpace.PSUM))
        psum_q = es.enter_context(
            tc.tile_pool(name="psum_q", bufs=2, space=bass.MemorySpace.PSUM))

        # xbar: x slots 0=halo_lo, 1..40 real, 41=halo_hi; yb col 40 =
        # pad(yb0), col 41 unused (even stride keeps bf16 2x alignment)
        xbar = state.tile([128, NXS + 2, NYB + 2], BF, tag="xbar")
        # qx on the 41-wide overlap slab (col j = global x s-1+j), no halos
        qx = state.tile([128, NXS + 1, NYB], BF, tag="qx")
        # qy: yb col 0 = pad(yb39), real yb at cols 1..40, col 41 unused
        qy = state.tile([128, NXS, NYB + 2], BF, tag="qy")
        qt = state.tile([128, NXS, NYB], BF, tag="qt")
        m = state.tile([128, NXS, NYB], BF, tag="m")
        cxn = state.tile([128, NXS, NYB], BF, tag="cxn")
        lamx = state.tile([128, NXS + 1, NYB], BF, tag="lamx")
        nlamx = state.tile([128, NXS + 1, NYB], BF, tag="nlamx")
        lamy = state.tile([128, NXS, NYB], BF, tag="lamy")
        nlamy = state.tile([128, NXS, NYB], BF, tag="nlamy")
        lamt = state.tile([128, NXS, NYB], BF, tag="lamt")
        nlamt = state.tile([128, NXS, NYB], BF, tag="nlamt")
        mskc = state.tile([128, 8, NYB], BF, tag="mskc")
        W = {n: state.tile([128, 128], BF, tag=n, name=f"w_{n}")
             for n in wnames}
        Wx = state.tile([128, 128], F32R, tag="w_ni32r", name="w_ni32r")

        nc.sync.dma_start(xbar[:, 1:41, 0:40], dp["xb0"][:])
        x0 = xpool.tile([128, NXS, NYB], F32R, tag="x", name="x_init")
        nc.sync.dma_start(x0[:], dp["x00"][:])
        zt = zpool.tile([128, NXS, NYB], BF, tag="z", name="z_init")
        nc.sync.dma_start(zt[:], dp["z00"][:])
        nc.sync.dma_start(m[:], dp["m0"][:])
        nc.sync.dma_start(cxn[:], dp["cxn"][:])
        for nm, tl in (("lamx", lamx), ("nlamx", nlamx), ("lamy", lamy),
                       ("nlamy", nlamy), ("lamt", lamt), ("nlamt", nlamt),
                       ("mskc", mskc)):
            nc.sync.dma_start(tl[:], dp[nm][:])
        for n in wnames:
            nc.sync.dma_start(W[n][:], dp[n][:])
        nc.sync.dma_start(Wx[:], dp["w_ni32r"][:])
        nc.vector.memset(qx[:], 0.0)
        nc.vector.memset(qy[:], 0.0)
        nc.vector.memset(qt[:], 0.0)
        nc.vector.tensor_copy(xbar[:, 1:41, 40:41], xbar[:, 1:41, 0:1])

        def exchange(round_idx):
            """AG of my (first,last) xbar planes; returns gathered dram tile."""
            bin_ = dpool.tile([2, 128, NYB], BF, tag="bin",
                              name=f"bin{round_idx}")
            bout = dpool.tile([8, 128, NYB], BF, tag="bout",
                              name=f"bout{round_idx}")
            nc.sync.dma_start(bin_[0], xbar[:, 1, 0:40])
            nc.sync.dma_start(bin_[1], xbar[:, 40, 0:40])
            nc.gpsimd.collective_compute(
                "AllGather", AX.bypass, replica_groups=GROUPS,
                ins=[bin_[:]], outs=[bout[:]])
            return bout

        def recv(bout):
            """DMA gathered planes to SBUF; one masked multiply + one
            axis-reduce -> (hi, lo), then 2 small copies into xbar halos."""
            gath = gpool.tile([128, 8, NYB], BF, tag="gath")
            nc.sync.dma_start(gath[:], bout[:].transpose([1, 0, 2]))
            u = gpool.tile([128, 8, NYB], BF, tag="u")
            nc.vector.tensor_tensor(u[:], gath[:], mskc[:], AX.mult)
            h = gpool.tile([128, 2, NYB], BF, tag="h")
            # view u as [p, e(2), y(40), j(4)] (slot = 2j+e) and reduce j
            uv = u[:].rearrange("p (j e) y -> p e y j", j=4, e=2)
            with nc.allow_low_precision(reason="one-hot select, sum exact"):
                nc.vector.tensor_reduce(h[:], uv, mybir.AxisListType.X,
                                        AX.add)
            nc.scalar.copy(xbar[:, 41, 0:40], h[:, 0, :])
            nc.scalar.copy(xbar[:, 0, 0:40], h[:, 1, :])

        bout = exchange(0)

        ac2 = float(np.float32(a_) * np.float32(c2))
        CORD = (0, 3, 1, 2)

        for k in range(T):
            # --- p-phase early (DVE; only needs xbar) ---
            t1 = spool.tile([128, NXS, NYB], BF, tag="t1")
            nc.vector.tensor_scalar(t1[:], xbar[:, 1:41, 0:40], ac2,
                                    None, AX.mult)
            nc.vector.tensor_sub(t1[:], t1[:], cxn[:])
            nc.vector.tensor_scalar(m[:], m[:], a_, None, AX.mult)
            nc.vector.tensor_add(m[:], m[:], t1[:])

            # --- qy/qt psums on PE ---
            ps_y, ps_t = {}, {}
            for c in CORD:
                sl = slice(1 + NCH * c, 1 + NCH * (c + 1))
                slq = slice(NCH * c, NCH * (c + 1))
                psy = psum_y.tile([128, NCH, NYB], F32, tag="psy",
                                  name=f"psy{k}_{c}")
                nc.tensor.matmul(psy[:], W["w_i"][:], qy[:, slq, 1:41],
                                 start=True, stop=False)
                nc.tensor.matmul(psy[:], W["w_dy"][:], xbar[:, sl, 0:40],
                                 start=False, stop=False)
                nc.tensor.matmul(psy[:], W["w_cy"][:], xbar[:, sl, 1:41],
                                 start=False, stop=True)
                ps_y[c] = psy
                pst = psum_t.tile([128, NCH, NYB], F32, tag="pst",
                                  name=f"pst{k}_{c}")
                nc.tensor.matmul(pst[:], W["w_i"][:], qt[:, slq, :],
                                 start=True, stop=False)
                nc.tensor.matmul(pst[:], W["w_dt"][:], xbar[:, sl, 0:40],
                                 start=False, stop=True)
                ps_t[c] = pst

            # --- qx via PE psums (c2-scaled x-shift stencil, incl halo
            # cols); chunks of 10|10|10|11 over the 41-wide slab. recv-
            # dependent chunks (0: halo_lo, 3: halo_hi) run last ---
            recv(bout)
            ps_q = {}
            for c in (1, 2, 0, 3):
                lo = 10 * c
                w = 11 if c == 3 else 10
                psq = psum_q.tile([128, 11, NYB], F32, tag="psq",
                                  name=f"psq{k}_{c}")
                pv = psq[:, 0:w, :]
                nc.tensor.matmul(pv, W["w_i"][:], qx[:, lo:lo + w, :],
                                 start=True, stop=False)
                nc.tensor.matmul(pv, W["w_ic2"][:],
                                 xbar[:, lo + 1:lo + w + 1, 0:40],
                                 start=False, stop=False)
                nc.tensor.matmul(pv, W["w_nic2"][:],
                                 xbar[:, lo:lo + w, 0:40],
                                 start=False, stop=True)
                ps_q[c] = (psq, lo, w)
            for c in (1, 2, 0, 3):
                psq, lo, w = ps_q[c]
                nc.vector.tensor_tensor(qx[:, lo:lo + w, :], psq[:, 0:w, :],
                                        nlamx[:, lo:lo + w, :], AX.max)
                nc.vector.tensor_tensor(qx[:, lo:lo + w, :],
                                        qx[:, lo:lo + w, :],
                                        lamx[:, lo:lo + w, :], AX.min)

            # --- per-chunk clips, then per-chunk x-psum + xbar writes:
            # pipelined so the PE restarts as soon as chunk deps land ---
            x1 = xpool.tile([128, NXS, NYB], F32R, tag="x", name=f"x{k}")
            xbp = spool.tile([128, NXS, NYB], BF, tag="xbp")
            zn = zpool.tile([128, NXS, NYB], BF, tag="z", name=f"z{k}")
            for c in CORD:
                slq = slice(NCH * c, NCH * (c + 1))
                nc.vector.tensor_tensor(qy[:, slq, 1:41], ps_y[c][:],
                                        nlamy[:, slq, :], AX.max)
                nc.vector.tensor_tensor(qy[:, slq, 1:41], qy[:, slq, 1:41],
                                        lamy[:, slq, :], AX.min)
                nc.scalar.copy(qy[:, slq, 0:1], qy[:, slq, 40:41])
                nc.vector.tensor_tensor(qt[:, slq, :], ps_t[c][:],
                                        nlamt[:, slq, :], AX.max)
                nc.vector.tensor_tensor(qt[:, slq, :], qt[:, slq, :],
                                        lamt[:, slq, :], AX.min)

            for c in CORD:
                slq = slice(NCH * c, NCH * (c + 1))          # qx[x-1]
                slq1 = slice(NCH * c + 1, NCH * (c + 1) + 1)  # qx[x]
                sl = slice(1 + NCH * c, 1 + NCH * (c + 1))
                ps = psum_x.tile([128, NCH, NYB], F32, tag="psx",
                               name=f"psx{k}_{c}")
                nc.tensor.matmul(ps[:], Wx[:], x0[:, slq, :],
                                 start=True, stop=False)
                nc.tensor.matmul(ps[:], W["w_dyh"][:], qy[:, slq, 1:41],
                                 start=False, stop=False)
                nc.tensor.matmul(ps[:], W["w_cyh"][:], qy[:, slq, 0:40],
                                 start=False, stop=False)
                nc.tensor.matmul(ps[:], W["w_dth"][:], qt[:, slq, :],
                                 start=False, stop=False)
                nc.tensor.matmul(ps[:], W["w_i"][:], qx[:, slq, :],
                                 start=False, stop=False)
                nc.tensor.matmul(ps[:], W["w_ni"][:], qx[:, slq1, :],
                                 start=False, stop=False)
                nc.tensor.matmul(ps[:], W["w_i"][:], m[:, slq, :],
                                 start=False, stop=True)
                if k < T - 1:
                    nc.scalar.activation(xbp[:, slq, :], ps[:], ACTF.Copy,
                                         scale=-(1.0 + th))
                    # xbar chunk + its pad-col slice ready immediately
                    nc.vector.tensor_sub(xbar[:, sl, 0:40], xbp[:, slq, :],
                                         zt[:, slq, :])
                    nc.scalar.copy(xbar[:, sl, 40:41], xbar[:, sl, 0:1])
                    nc.scalar.activation(zn[:, slq, :], ps[:], ACTF.Copy,
                                         scale=-th)
                nc.scalar.activation(x1[:, slq, :], ps[:], ACTF.Copy,
                                     scale=-1.0)
                if k < T - 1 and c == 3:
                    bout = exchange(k + 1)
            x0 = x1
            zt = zn

        # out = c2 * X1 (c2 folded host-side via out scaling input? no:
        # scale here with ACT once)
        nc.sync.dma_start(out_dram[:], x0[:].bitcast(F32))

    nc.compile()
    return nc


@lru_cache(maxsize=4)
def _compiled(scalars, T):
    return _build_nc(scalars, T)


def _make_in_maps(x, lambda_map, scalars, sig):
    import ml_dtypes
    bf = ml_dtypes.bfloat16
    stats = _stationaries()
    a_, th, c2 = scalars
    rc2 = 1.0 / np.float32(c2)
    in_maps = []
    for rank in range(8):
        mbi, pos = rank // 4, rank % 4
        s = pos * NXS
        xs = slice(s, s + NXS)
        xn = np.ascontiguousarray(x[mbi, 0, xs]).astype(np.float32)
        X = xn * rc2
        lam = lambda_map[mbi].astype(np.float32) / np.float32(sig)
        # x-channel lambda on the 41-wide overlap slab [s-1, s+40)
        idx = [(s - 1 + j) % 160 for j in range(NXS + 1)]
        lx = lam[0][idx]
        nxt, prv = (pos + 1) % 4, (pos - 1) % 4
        mc = np.zeros((128, 8, NYB), np.float32)
        mc[:, 2 * nxt, :] = 1.0      # next's first plane -> halo_hi (even)
        mc[:, 2 * prv + 1, :] = 1.0  # prev's last plane  -> halo_lo (odd)
        mm = dict(
            xb0=to_dev(X).astype(bf),
            x00=to_dev(X),
            z00=to_dev(np.float32(th) * X).astype(bf),
            m0=to_dev(xn / np.float32(sig)).astype(bf),
            cxn=to_dev(np.float32(a_) * xn).astype(bf),
            lamx=to_dev(lx).astype(bf), nlamx=to_dev(-lx).astype(bf),
            lamy=to_dev(lam[1][xs]).astype(bf),
            nlamy=to_dev(-lam[1][xs]).astype(bf),
            lamt=to_dev(lam[2][xs]).astype(bf),
            nlamt=to_dev(-lam[2][xs]).astype(bf),
            mskc=mc.astype(bf),
        )
        for k2, v in stats.items():
            # forward y/t stencils carry the c2 gradient coefficient
            if k2 in ("w_dy", "w_cy", "w_dt"):
                v = v * np.float32(c2)
            mm[k2] = v.astype(bf)
        eye = np.eye(128, dtype=np.float32)
        mm["w_ic2"] = (np.float32(c2) * eye).astype(bf)
        mm["w_nic2"] = (-np.float32(c2) * eye).astype(bf)
        mm["w_ni32r"] = -np.eye(128, dtype=np.float32)
        in_maps.append(mm)
    return in_maps


def kernel(x, lambda_map, tau, sigma, theta):
    x = np.asarray(x, dtype=np.float32)
    lambda_map = np.asarray(lambda_map, dtype=np.float32)
    L = math.sqrt(13.0)
    sig = float(1.0 / (1.0 + math.exp(-float(np.asarray(sigma)[0])))) / L
    ta = float(1.0 / (1.0 + math.exp(-float(np.asarray(tau)[0])))) / L
    th = float(1.0 / (1.0 + math.exp(-float(np.asarray(theta)[0]))))
    a_ = 1.0 / (1.0 + sig)
    c2 = ta * sig
    scalars = tuple(float(np.float32(v)) for v in (a_, th, c2))

    nc = _compiled(scalars, T_ITERS)
    in_maps = _make_in_maps(x, lambda_map, scalars, sig)
    res = run_bass_kernel_spmd(nc, in_maps, core_ids=list(range(8)),
                               trace=TRACE)
    global _LAST_RESULTS
    _LAST_RESULTS = res

    out = np.zeros((2, 1, 160, 160, 32), np.float32)
    for rank in range(8):
        mbi, pos = rank // 4, rank % 4
        s = pos * NXS
        out[mbi, 0, s:s + NXS] = from_dev(
            res.results[rank]["out"]) * np.float32(c2)
    return out
